# revision 22
# baseline (speedup 1.0000x reference)
# Trainium2 Bass kernel for nn_DecomposeNetwork (vq_codebook).
#
# Reference computation (per modality x in {m, a}):
#   h   = relu(x @ W_in + b)                       [B, 500]
#   z_p = enc_own(h)    (500->500->2000->64, relu on first two)
#   z_s = enc_shared(h) (same shapes)
#   loss term from VQ distances of z_s vs codebook [4096, 64]
#   z   = z_p + z_s
#   xh  = dec_head(dec_shared(z))                  [B, 2000]
# Returned: ((z_m_p, z_m_s, z_a_p, z_a_s), (x_m_hat, x_a_hat), loss_code)
#
# Key simplification: the quantized vectors / soft-assignment q are dead code
# for the returned outputs.  loss = 1.25/(B*64) * sum_b max_c d[b,c] where
# d[b,c] = ||z_b||^2 + ||cb_c||^2 - 2 z_b.cb_c  (argmax ties don't change the
# max value).  We compute max_c(||cb_c||^2 - 2 z.cb) via a matmul with an
# augmented codebook row, plus sum ||z||^2, accumulated per-core and reduced
# on the host (the only cross-device term).
#
# Sharding: cores 0-3 process x_m (4096 rows each), cores 4-7 process x_a.
# The modality split halves the per-core weight footprint so ALL weights stay
# resident in SBUF for the whole kernel.  All cores run one SPMD program;
# per-core differences are input bindings only.
#
# Layout: activations are feature-major [features(partitions), batch(free)];
# weights are host-pre-tiled to [128, k_tiles, M] so each [128,128] stationary
# tile is a direct slice.  Dims padded: 2000->2048, 500->512.  Matmuls in
# bf16 with fp32 PSUM accumulation; outputs stored fp32.

import numpy as np
import ml_dtypes

import concourse.bass as bass
import concourse.tile as tile
from concourse import bacc
from concourse import mybir
from concourse import bass_utils

# 16-bit matmul dtype: fp16 has 10 mantissa bits vs bf16's 7 (~8x lower
# rounding error); all tensors here are O(1-10) so fp16 range is ample and
# the PE rate is identical for both 16-bit formats.
BF16 = np.float16

P = 128
NB = 512          # batch columns per chunk (matmul free dim / PSUM bank)
B_FULL = 16384
N_CORES = 8
CORES_PER_MOD = 4
B_CORE = B_FULL // CORES_PER_MOD   # 4096 rows per core

NX = 2000         # raw input features
KX = 2048         # padded input features  (16 k-tiles)
NH = 500
H = 512           # padded hidden          (4 tiles)
NE3 = 2000
E3 = 2048         # padded encoder L2 out  (16 tiles)
NZ = 64
C = 4096          # codebook entries
CT = C // NB      # codebook column chunks (8)
NOUT = 2000
OUT = 2048        # padded output features (16 tiles)
BETA = 0.25

F32 = mybir.dt.float32
BF16_DT = mybir.dt.float16

# name -> (k_tiles, M) for the [128, k_tiles, M] weight layout
WEIGHT_SPECS = {
    "w_in":  (KX // P, H),
    "w_e1o": (H // P, H),
    "w_e2o": (H // P, E3),
    "w_e3o": (E3 // P, NZ),
    "w_e1s": (H // P, H),
    "w_e2s": (H // P, E3),
    "w_e3s": (E3 // P, NZ),
    "w_d1":  (1, E3),          # K=64 zero-padded to 128
    "w_d2":  (E3 // P, H),
    "w_d3":  (H // P, H),
    "w_hd":  (H // P, OUT),
}
# name -> (partitions, m_tiles)
BIAS_SPECS = {
    "b_in":  (P, H // P),
    "b_e1o": (P, H // P),
    "b_e2o": (P, E3 // P),
    "b_e3o": (NZ, 1),
    "b_e1s": (P, H // P),
    "b_e2s": (P, E3 // P),
    "b_e3s": (NZ, 1),
    "b_d1":  (P, E3 // P),
    "b_d2":  (P, H // P),
    "b_d3":  (P, H // P),
    "b_hd":  (P, OUT // P),
}


def build_program(b_core=B_CORE, num_devices=N_CORES, repeat=1):
    nc = bacc.Bacc("TRN2", debug=False, num_devices=num_devices)

    x_T = nc.dram_tensor("x_T", [KX, b_core], BF16_DT, kind="ExternalInput").ap()
    w = {
        name: nc.dram_tensor(name, [P, kt, m], BF16_DT, kind="ExternalInput").ap()
        for name, (kt, m) in WEIGHT_SPECS.items()
    }
    b = {
        name: nc.dram_tensor(name, [pp, mt], F32, kind="ExternalInput").ap()
        for name, (pp, mt) in BIAS_SPECS.items()
    }
    cb_aug = nc.dram_tensor("cb_aug", [P, C], BF16_DT, kind="ExternalInput").ap()

    z_p_T = nc.dram_tensor("z_p_T", [NZ, b_core], F32, kind="ExternalOutput").ap()
    z_s_T = nc.dram_tensor("z_s_T", [NZ, b_core], F32, kind="ExternalOutput").ap()
    xhat_T = nc.dram_tensor("xhat_T", [OUT, b_core], F32, kind="ExternalOutput").ap()
    acc_max_d = nc.dram_tensor("acc_max", [P, 1], F32, kind="ExternalOutput").ap()
    acc_zsq_d = nc.dram_tensor("acc_zsq", [NZ, 1], F32, kind="ExternalOutput").ap()

    with tile.TileContext(nc) as tc:
        _emit(tc, b_core, x_T, w, b, cb_aug, z_p_T, z_s_T, xhat_T,
              acc_max_d, acc_zsq_d, repeat=repeat)
    nc.compile()
    return nc


def _emit(tc, b_core, x_T, w_d, b_d, cb_d, z_p_T, z_s_T, xhat_T,
          acc_max_d, acc_zsq_d, repeat=1):
    nc = tc.nc
    nchunk = b_core // NB
    x_R = x_T.rearrange("(kt p) b -> p kt b", p=P)
    Relu = mybir.ActivationFunctionType.Relu
    Ident = mybir.ActivationFunctionType.Identity
    Add = mybir.AluOpType.add
    Max = mybir.AluOpType.max
    AX = mybir.AxisListType.X
    KSUB = 4

    with (
        tc.tile_pool(name="singles", bufs=1) as singles,
        tc.tile_pool(name="xin", bufs=4) as xin,
        tc.tile_pool(name="hp", bufs=2) as hp,
        tc.tile_pool(name="mid", bufs=3) as mid,
        tc.tile_pool(name="big", bufs=2) as big,
        tc.tile_pool(name="zp", bufs=3) as zp,
        tc.tile_pool(name="zmisc", bufs=2) as zmisc,
        tc.tile_pool(name="hout", bufs=3) as hout,
        tc.tile_pool(name="qp", bufs=4) as qp,
        tc.tile_pool(name="psum", bufs=4, space="PSUM") as psum,
        tc.tile_pool(name="psin", bufs=4, space="PSUM") as psin,
    ):
        # ---- resident weights / biases ----
        # The SP HWDGE ring is in-order, so DMA emission order is queue
        # order: first-chunk inputs (w_in, b_in, x chunk 0) go first so PE
        # can start early; remaining weights stream in usage order.
        w = {}
        bias = {}

        def load_w(name):
            ap = w_d[name]
            t = singles.tile(list(ap.shape), BF16_DT, name=f"sb_{name}")
            nc.sync.dma_start(out=t, in_=ap)
            w[name] = t

        def load_b(name):
            ap = b_d[name]
            t = singles.tile(list(ap.shape), F32, name=f"sb_{name}")
            nc.sync.dma_start(out=t, in_=ap)
            bias[name] = t

        # w_in split 4 ways: the very first LDWEIGHTS then waits on a
        # 0.5MB DMA instead of the full 2MB
        w_in_parts = []
        for i in range(4):
            t = singles.tile([P, 4, H], BF16_DT, name=f"sb_w_in_{i}")
            nc.sync.dma_start(out=t, in_=w_d["w_in"][:, 4 * i:4 * (i + 1), :])
            w_in_parts.append(t)
        load_b("b_in")
        x0_tiles = []
        for kb in range(KX // P // KSUB):
            xt = xin.tile([P, KSUB, NB], BF16_DT, tag="xt", name="xt")
            nc.sync.dma_start(out=xt, in_=x_R[:, kb * KSUB:(kb + 1) * KSUB, 0:NB])
            x0_tiles.append(xt)
        preloaded = {0: x0_tiles}
        for name in ("b_e1s", "b_e2s", "b_e3s", "b_e1o", "b_e2o", "b_e3o",
                     "b_d1", "b_d2", "b_d3", "b_hd"):
            load_b(name)
        for name in ("w_e1s", "w_e2s", "w_e1o", "w_e2o", "w_e3s", "w_e3o"):
            load_w(name)
        cb = singles.tile([P, C], BF16_DT, name="sb_cb")
        nc.sync.dma_start(out=cb, in_=cb_d)
        for name in ("w_d1", "w_d2", "w_d3", "w_hd"):
            load_w(name)

        acc_max = singles.tile([P, 1], F32, name="acc_max_sb")
        acc_zsq = singles.tile([NZ, 1], F32, name="acc_zsq_sb")
        nc.vector.memset(acc_max, 0.0)
        nc.vector.memset(acc_zsq, 0.0)

        ctr = [0]

        def copy_act(dst, src, bias_ap, relu):
            # psum -> sbuf with per-partition bias (+ optional relu), engine
            # alternated between ACT and DVE so neither becomes the bottleneck
            if ctr[0] % 2 == 0:
                nc.scalar.activation(dst, src, Relu if relu else Ident,
                                     bias=bias_ap)
            else:
                if relu:
                    nc.vector.tensor_scalar(dst, src, bias_ap, 0.0, Add, Max)
                else:
                    nc.vector.tensor_scalar(dst, src, bias_ap, None, Add)
            ctr[0] += 1

        def layer(dst, mt, src, kt, wname, bname, relu):
            # dst [128, mt, NB] sbuf <- act(w.T @ src + b); src [128, kt, NB]
            wt, bt = w[wname], bias[bname]
            for m in range(mt):
                ps = psum.tile([P, NB], F32, tag="ps", name="ps")
                for k in range(kt):
                    nc.tensor.matmul(ps, wt[:, k, m * P:(m + 1) * P],
                                     src[:, k, :], start=(k == 0),
                                     stop=(k == kt - 1))
                copy_act(dst[:, m, :], ps, bt[:, m:m + 1], relu)

        def enc12(h, sfx):
            e1 = mid.tile([P, H // P, NB], BF16_DT, tag="mid", name="e1")
            layer(e1, H // P, h, H // P, f"w_e1{sfx}", f"b_e1{sfx}", True)
            e2 = big.tile([P, E3 // P, NB], BF16_DT, tag="big", name="e2")
            layer(e2, E3 // P, e1, H // P, f"w_e2{sfx}", f"b_e2{sfx}", True)
            return e2

        def enc3_packed(e2s, e2o):
            # Both final 2048->64 linears have M=64, so they run packed in
            # disjoint PE column halves (tile_position col groups) and their
            # matmul pairs execute concurrently: ~16 matmul slots instead
            # of 32.  Separate psum tiles keep the accumulation groups
            # independent; the (0,64) output must sit at base partition 64.
            ps_s = psum.tile([P, NB], F32, tag="ps", name="ps")
            ps_o = psum.tile([P, NB], F32, tag="ps", name="ps")
            ws, wo = w["w_e3s"], w["w_e3o"]
            for k in range(E3 // P):
                nc.tensor.matmul(ps_s[:NZ, :], ws[:, k, :], e2s[:, k, :],
                                 start=(k == 0), stop=(k == E3 // P - 1),
                                 tile_position=(0, 0))
                nc.tensor.matmul(ps_o[NZ:2 * NZ, :], wo[:, k, :], e2o[:, k, :],
                                 start=(k == 0), stop=(k == E3 // P - 1),
                                 tile_position=(0, NZ))
            z_s = zp.tile([NZ, NB], F32, tag="z", name="z_s")
            copy_act(z_s, ps_s[:NZ, :], bias["b_e3s"][:, 0:1], False)
            z_p = zp.tile([NZ, NB], F32, tag="z", name="z_p")
            copy_act(z_p, ps_o[NZ:2 * NZ, :], bias["b_e3o"][:, 0:1], False)
            return z_s, z_p

        chunk_list = [cc for _ in range(repeat) for cc in range(nchunk)]
        for ci_, c in enumerate(chunk_list):
            last_chunk = ci_ == len(chunk_list) - 1
            # stores go via SWDGE so the in-order SP (load) ring is never
            # blocked behind them; last chunk uses the now-idle SP ring to
            # shorten the kernel tail drain
            st_eng = nc.sync if last_chunk else nc.gpsimd
            bs = slice(c * NB, (c + 1) * NB)

            # ---- input layer: stream x in 4 batched sub-tiles, 4 dedicated
            # psum banks (pool psin) accumulate over all 16 k-tiles ----
            pss = [psin.tile([P, NB], F32, tag="psi", name="psi")
                   for _ in range(H // P)]
            pre = preloaded.pop(c, None)
            for kb in range(KX // P // KSUB):
                if pre is not None:
                    xt = pre[kb]
                else:
                    xt = xin.tile([P, KSUB, NB], BF16_DT, tag="xt", name="xt")
                    nc.sync.dma_start(out=xt,
                                      in_=x_R[:, kb * KSUB:(kb + 1) * KSUB, bs])
                for j in range(KSUB):
                    k = kb * KSUB + j
                    for m in range(H // P):
                        nc.tensor.matmul(pss[m],
                                         w_in_parts[k // 4][:, k % 4,
                                                            m * P:(m + 1) * P],
                                         xt[:, j, :], start=(k == 0),
                                         stop=(k == KX // P - 1))
            h = hp.tile([P, H // P, NB], BF16_DT, tag="h", name="h")
            for m in range(H // P):
                copy_act(h[:, m, :], pss[m], bias["b_in"][:, m:m + 1], True)

            # ---- encoders: e1/e2 of both, then the packed e3 pair ----
            e2s = enc12(h, "s")
            e2o = enc12(h, "o")
            z_s, z_p = enc3_packed(e2s, e2o)
            st_eng.dma_start(out=z_s_T[:, bs], in_=z_s)
            st_eng.dma_start(out=z_p_T[:, bs], in_=z_p)

            z_aug = zmisc.tile([P, NB], BF16_DT, tag="zaug", name="z_aug")
            nc.vector.memset(z_aug[NZ:, :], 0.0)
            nc.vector.memset(z_aug[NZ:NZ + 1, :], 1.0)
            nc.vector.tensor_scalar_mul(z_aug[:NZ, :], z_s, -2.0)
            lm = qp.tile([P, NB // P, CT], F32, tag="lm", name="lm")
            zsq = zmisc.tile([NZ, NB], F32, tag="zsq", name="zsq")
            nc.vector.tensor_mul(zsq, z_s, z_s)
            rs = qp.tile([NZ, 1], F32, tag="rs", name="rs")
            nc.vector.reduce_sum(rs, zsq, axis=AX)
            nc.vector.tensor_add(acc_zsq, acc_zsq, rs)

            quant_list = [(mb, qq) for mb in range(NB // P) for qq in range(CT)]
            qi = [0]

            def emit_quant(n):
                # VQ distance matmul + DVE rowmax; each psum drains via a
                # DVE reduce (~3x a matmul) so these are spread as PE filler
                for _ in range(n):
                    if qi[0] >= len(quant_list):
                        return
                    mb, qq = quant_list[qi[0]]
                    qi[0] += 1
                    qps = psum.tile([P, NB], F32, tag="ps", name="ps")
                    nc.tensor.matmul(qps, z_aug[:, mb * P:(mb + 1) * P],
                                     cb[:, qq * NB:(qq + 1) * NB],
                                     start=True, stop=True)
                    nc.vector.reduce_max(lm[:, mb, qq:qq + 1], qps, axis=AX)

            # fill the z -> z_sum -> d1 dependency latency with VQ matmuls
            emit_quant(6)

            # ---- decoder ----
            z_sum = zmisc.tile([P, NB], BF16_DT, tag="zsum", name="z_sum")
            nc.vector.memset(z_sum[NZ:, :], 0.0)
            nc.vector.tensor_add(z_sum[:NZ, :], z_p, z_s)

            d1 = big.tile([P, E3 // P, NB], BF16_DT, tag="big", name="d1")
            wt, bt = w["w_d1"], bias["b_d1"]
            for m in range(E3 // P):
                ps = psum.tile([P, NB], F32, tag="ps", name="ps")
                nc.tensor.matmul(ps, wt[:, 0, m * P:(m + 1) * P], z_sum,
                                 start=True, stop=True)
                copy_act(d1[:, m, :], ps, bt[:, m:m + 1], True)
                if m % 3 == 2:
                    emit_quant(1)

            # ---- d2 matmuls interleaved with remaining VQ matmuls ----
            d2 = mid.tile([P, H // P, NB], BF16_DT, tag="mid", name="d2")
            wt, bt = w["w_d2"], bias["b_d2"]
            for m in range(H // P):
                ps = psum.tile([P, NB], F32, tag="ps", name="ps")
                for k in range(E3 // P):
                    nc.tensor.matmul(ps, wt[:, k, m * P:(m + 1) * P],
                                     d1[:, k, :], start=(k == 0),
                                     stop=(k == E3 // P - 1))
                    if k % 2 == 1:
                        emit_quant(1)
                copy_act(d2[:, m, :], ps, bt[:, m:m + 1], True)
            assert qi[0] == len(quant_list)
            for mb in range(NB // P):
                rm = qp.tile([P, 1], F32, tag="rm", name="rm")
                nc.vector.reduce_max(rm, lm[:, mb, :], axis=AX)
                nc.vector.tensor_add(acc_max, acc_max, rm)

            d3 = mid.tile([P, H // P, NB], BF16_DT, tag="mid", name="d3")
            layer(d3, H // P, d2, H // P, "w_d3", "b_d3", True)

            # ---- head: fp32 out, DMA per m-tile ----
            wt, bt = w["w_hd"], bias["b_hd"]
            for m in range(OUT // P):
                ps = psum.tile([P, NB], F32, tag="ps", name="ps")
                for k in range(H // P):
                    nc.tensor.matmul(ps, wt[:, k, m * P:(m + 1) * P],
                                     d3[:, k, :], start=(k == 0),
                                     stop=(k == H // P - 1))
                ho = hout.tile([P, NB], F32, tag="ho", name="ho")
                copy_act(ho, ps, bt[:, m:m + 1], False)
                st_eng.dma_start(out=xhat_T[m * P:(m + 1) * P, bs], in_=ho)

        nc.sync.dma_start(out=acc_max_d, in_=acc_max)
        nc.sync.dma_start(out=acc_zsq_d, in_=acc_zsq)


# ---------------- host-side data prep ----------------

def _tile_w(W, Kp, Mp):
    W = np.asarray(W, np.float32)
    K, M = W.shape
    Wp = np.zeros((Kp, Mp), np.float32)
    Wp[:K, :M] = W
    return Wp.reshape(Kp // P, P, Mp).transpose(1, 0, 2).astype(BF16)


def _tile_b(bv, Mp):
    bv = np.asarray(bv, np.float32)
    bp = np.zeros((Mp,), np.float32)
    bp[:bv.shape[0]] = bv
    if Mp < P:
        return np.ascontiguousarray(bp.reshape(1, Mp).T)
    return np.ascontiguousarray(bp.reshape(Mp // P, P).T)


def _prep_mod_inputs(params, modality):
    own_in = params["in_m"] if modality == "m" else params["in_a"]
    enc_own = params["enc_main"] if modality == "m" else params["enc_aux"]
    enc_sh = params["enc_shared"]
    dec = params["dec_shared"]
    head = params["dec_m"] if modality == "m" else params["dec_a"]
    cb = np.asarray(params["codebook"], np.float32)

    m = {}
    m["w_in"], m["b_in"] = _tile_w(own_in[0], KX, H), _tile_b(own_in[1], H)
    for sfx, enc in (("o", enc_own), ("s", enc_sh)):
        m[f"w_e1{sfx}"] = _tile_w(enc[0][0], H, H)
        m[f"b_e1{sfx}"] = _tile_b(enc[0][1], H)
        m[f"w_e2{sfx}"] = _tile_w(enc[1][0], H, E3)
        m[f"b_e2{sfx}"] = _tile_b(enc[1][1], E3)
        m[f"w_e3{sfx}"] = _tile_w(enc[2][0], E3, NZ)
        m[f"b_e3{sfx}"] = _tile_b(enc[2][1], NZ)
    m["w_d1"], m["b_d1"] = _tile_w(dec[0][0], P, E3), _tile_b(dec[0][1], E3)
    m["w_d2"], m["b_d2"] = _tile_w(dec[1][0], E3, H), _tile_b(dec[1][1], H)
    m["w_d3"], m["b_d3"] = _tile_w(dec[2][0], H, H), _tile_b(dec[2][1], H)
    m["w_hd"], m["b_hd"] = _tile_w(head[0], H, OUT), _tile_b(head[1], OUT)

    cb_aug = np.zeros((P, C), np.float32)
    cb_aug[:NZ, :] = cb.T
    cb_aug[NZ, :] = (cb * cb).sum(axis=1)
    m["cb_aug"] = cb_aug.astype(BF16)
    return m


def _prep_x(x, shard):
    xs = np.asarray(x[shard * B_CORE:(shard + 1) * B_CORE], np.float32)
    xp = np.zeros((KX, B_CORE), np.float32)
    xp[:NX, :] = xs.T
    return xp.astype(BF16)


_CACHE = {}


def _get_program():
    if "nc" not in _CACHE:
        _CACHE["nc"] = build_program()
    return _CACHE["nc"]


def build_in_maps(x_m, x_a, params):
    mod_inputs = {mod: _prep_mod_inputs(params, mod) for mod in ("m", "a")}
    in_maps = []
    for core in range(N_CORES):
        mod = "m" if core < CORES_PER_MOD else "a"
        im = dict(mod_inputs[mod])
        im["x_T"] = _prep_x(x_m if mod == "m" else x_a, core % CORES_PER_MOD)
        in_maps.append(im)
    return in_maps


def gather(outs):
    # outs: list (per core) of dicts name -> np.ndarray
    def gather_z(name, lo, hi):
        return np.concatenate(
            [outs[c][name].T for c in range(lo, hi)], axis=0).astype(np.float32)

    z_m_p = gather_z("z_p_T", 0, CORES_PER_MOD)
    z_m_s = gather_z("z_s_T", 0, CORES_PER_MOD)
    z_a_p = gather_z("z_p_T", CORES_PER_MOD, N_CORES)
    z_a_s = gather_z("z_s_T", CORES_PER_MOD, N_CORES)
    x_m_hat = np.concatenate(
        [outs[c]["xhat_T"][:NOUT].T for c in range(CORES_PER_MOD)], axis=0)
    x_a_hat = np.concatenate(
        [outs[c]["xhat_T"][:NOUT].T for c in range(CORES_PER_MOD, N_CORES)],
        axis=0)
    total = 0.0
    for c in range(N_CORES):
        total += float(outs[c]["acc_max"].sum()) + float(outs[c]["acc_zsq"].sum())
    loss = np.float32((1.0 + BETA) / (B_FULL * NZ) * total)

    return ((z_m_p, z_m_s, z_a_p, z_a_s),
            (np.ascontiguousarray(x_m_hat), np.ascontiguousarray(x_a_hat)),
            loss)


def run(x_m, x_a, params, trace=False):
    nc = _get_program()
    in_maps = build_in_maps(x_m, x_a, params)
    res = bass_utils.run_bass_kernel_spmd(
        nc, in_maps, core_ids=list(range(N_CORES)), trace=trace)
    return gather(res.results), res


def kernel(x_m, x_a, params):
    result, _ = run(x_m, x_a, params)
    return result


# revision 23
# speedup vs baseline: 5.7321x; 5.7321x over previous
# Trainium2 Bass kernel for nn_DecomposeNetwork (vq_codebook).
#
# Reference computation (per modality x in {m, a}):
#   h   = relu(x @ W_in + b)                       [B, 500]
#   z_p = enc_own(h)    (500->500->2000->64, relu on first two)
#   z_s = enc_shared(h) (same shapes)
#   loss term from VQ distances of z_s vs codebook [4096, 64]
#   z   = z_p + z_s
#   xh  = dec_head(dec_shared(z))                  [B, 2000]
# Returned: ((z_m_p, z_m_s, z_a_p, z_a_s), (x_m_hat, x_a_hat), loss_code)
#
# Key simplification: the quantized vectors / soft-assignment q are dead code
# for the returned outputs.  loss = 1.25/(B*64) * sum_b max_c d[b,c] where
# d[b,c] = ||z_b||^2 + ||cb_c||^2 - 2 z_b.cb_c  (argmax ties don't change the
# max value).  We compute max_c(||cb_c||^2 - 2 z.cb) via a matmul with an
# augmented codebook row, plus sum ||z||^2, accumulated per-core and reduced
# on the host (the only cross-device term).
#
# Sharding: cores 0-3 process x_m (4096 rows each), cores 4-7 process x_a.
# The modality split halves the per-core weight footprint so ALL weights stay
# resident in SBUF for the whole kernel.  All cores run one SPMD program;
# per-core differences are input bindings only.
#
# Layout: activations are feature-major [features(partitions), batch(free)];
# weights are host-pre-tiled to [128, k_tiles, M] so each [128,128] stationary
# tile is a direct slice.  Dims padded: 2000->2048, 500->512.  Matmuls in
# bf16 with fp32 PSUM accumulation; outputs stored fp32.

import numpy as np
import ml_dtypes

import concourse.bass as bass
import concourse.tile as tile
from concourse import bacc
from concourse import mybir
from concourse import bass_utils

BF16 = ml_dtypes.bfloat16

P = 128
NB = 512          # batch columns per chunk (matmul free dim / PSUM bank)
B_FULL = 16384
N_CORES = 8
CORES_PER_MOD = 4
B_CORE = B_FULL // CORES_PER_MOD   # 4096 rows per core

NX = 2000         # raw input features
KX = 2048         # padded input features  (16 k-tiles)
NH = 500
H = 512           # padded hidden          (4 tiles)
NE3 = 2000
E3 = 2048         # padded encoder L2 out  (16 tiles)
NZ = 64
C = 4096          # codebook entries
CT = C // NB      # codebook column chunks (8)
NOUT = 2000
OUT = 2048        # padded output features (16 tiles)
BETA = 0.25

F32 = mybir.dt.float32
BF16_DT = mybir.dt.bfloat16

# name -> (k_tiles, M) for the [128, k_tiles, M] weight layout
WEIGHT_SPECS = {
    "w_in":  (KX // P, H),
    "w_e1o": (H // P, H),
    "w_e2o": (H // P, E3),
    "w_e3o": (E3 // P, NZ),
    "w_e1s": (H // P, H),
    "w_e2s": (H // P, E3),
    "w_e3s": (E3 // P, NZ),
    "w_d1":  (1, E3),          # K=64 zero-padded to 128
    "w_d2":  (E3 // P, H),
    "w_d3":  (H // P, H),
    "w_hd":  (H // P, OUT),
}
# name -> (partitions, m_tiles)
BIAS_SPECS = {
    "b_in":  (P, H // P),
    "b_e1o": (P, H // P),
    "b_e2o": (P, E3 // P),
    "b_e3o": (NZ, 1),
    "b_e1s": (P, H // P),
    "b_e2s": (P, E3 // P),
    "b_e3s": (NZ, 1),
    "b_d1":  (P, E3 // P),
    "b_d2":  (P, H // P),
    "b_d3":  (P, H // P),
    "b_hd":  (P, OUT // P),
}


def build_program(b_core=B_CORE, num_devices=N_CORES, repeat=1):
    nc = bacc.Bacc("TRN2", debug=False, num_devices=num_devices)

    x_T = nc.dram_tensor("x_T", [KX, b_core], BF16_DT, kind="ExternalInput").ap()
    w = {
        name: nc.dram_tensor(name, [P, kt, m], BF16_DT, kind="ExternalInput").ap()
        for name, (kt, m) in WEIGHT_SPECS.items()
    }
    b = {
        name: nc.dram_tensor(name, [pp, mt], F32, kind="ExternalInput").ap()
        for name, (pp, mt) in BIAS_SPECS.items()
    }
    cb_aug = nc.dram_tensor("cb_aug", [P, C], BF16_DT, kind="ExternalInput").ap()

    z_p_T = nc.dram_tensor("z_p_T", [NZ, b_core], F32, kind="ExternalOutput").ap()
    z_s_T = nc.dram_tensor("z_s_T", [NZ, b_core], F32, kind="ExternalOutput").ap()
    xhat_T = nc.dram_tensor("xhat_T", [OUT, b_core], F32, kind="ExternalOutput").ap()
    acc_max_d = nc.dram_tensor("acc_max", [P, 1], F32, kind="ExternalOutput").ap()
    acc_zsq_d = nc.dram_tensor("acc_zsq", [NZ, 1], F32, kind="ExternalOutput").ap()

    with tile.TileContext(nc) as tc:
        _emit(tc, b_core, x_T, w, b, cb_aug, z_p_T, z_s_T, xhat_T,
              acc_max_d, acc_zsq_d, repeat=repeat)
    nc.compile()
    return nc


def _emit(tc, b_core, x_T, w_d, b_d, cb_d, z_p_T, z_s_T, xhat_T,
          acc_max_d, acc_zsq_d, repeat=1):
    nc = tc.nc
    nchunk = b_core // NB
    x_R = x_T.rearrange("(kt p) b -> p kt b", p=P)
    Relu = mybir.ActivationFunctionType.Relu
    Ident = mybir.ActivationFunctionType.Identity
    Add = mybir.AluOpType.add
    Max = mybir.AluOpType.max
    AX = mybir.AxisListType.X
    KSUB = 4

    with (
        tc.tile_pool(name="singles", bufs=1) as singles,
        tc.tile_pool(name="xin", bufs=4) as xin,
        tc.tile_pool(name="hp", bufs=2) as hp,
        tc.tile_pool(name="mid", bufs=3) as mid,
        tc.tile_pool(name="big", bufs=2) as big,
        tc.tile_pool(name="zp", bufs=3) as zp,
        tc.tile_pool(name="zmisc", bufs=2) as zmisc,
        tc.tile_pool(name="hout", bufs=3) as hout,
        tc.tile_pool(name="qp", bufs=4) as qp,
        tc.tile_pool(name="psum", bufs=4, space="PSUM") as psum,
        tc.tile_pool(name="psin", bufs=4, space="PSUM") as psin,
    ):
        # ---- resident weights / biases ----
        # The SP HWDGE ring is in-order, so DMA emission order is queue
        # order: first-chunk inputs (w_in, b_in, x chunk 0) go first so PE
        # can start early; remaining weights stream in usage order.
        w = {}
        bias = {}

        def load_w(name):
            ap = w_d[name]
            t = singles.tile(list(ap.shape), BF16_DT, name=f"sb_{name}")
            nc.sync.dma_start(out=t, in_=ap)
            w[name] = t

        def load_b(name):
            ap = b_d[name]
            t = singles.tile(list(ap.shape), F32, name=f"sb_{name}")
            nc.sync.dma_start(out=t, in_=ap)
            bias[name] = t

        # w_in split 4 ways: the very first LDWEIGHTS then waits on a
        # 0.5MB DMA instead of the full 2MB
        w_in_parts = []
        for i in range(4):
            t = singles.tile([P, 4, H], BF16_DT, name=f"sb_w_in_{i}")
            nc.sync.dma_start(out=t, in_=w_d["w_in"][:, 4 * i:4 * (i + 1), :])
            w_in_parts.append(t)
        load_b("b_in")
        x0_tiles = []
        for kb in range(KX // P // KSUB):
            xt = xin.tile([P, KSUB, NB], BF16_DT, tag="xt", name="xt")
            nc.sync.dma_start(out=xt, in_=x_R[:, kb * KSUB:(kb + 1) * KSUB, 0:NB])
            x0_tiles.append(xt)
        preloaded = {0: x0_tiles}
        for name in ("b_e1s", "b_e2s", "b_e3s", "b_e1o", "b_e2o", "b_e3o",
                     "b_d1", "b_d2", "b_d3", "b_hd"):
            load_b(name)
        for name in ("w_e1s", "w_e2s", "w_e1o", "w_e2o", "w_e3s", "w_e3o"):
            load_w(name)
        cb = singles.tile([P, C], BF16_DT, name="sb_cb")
        nc.sync.dma_start(out=cb, in_=cb_d)
        for name in ("w_d1", "w_d2", "w_d3", "w_hd"):
            load_w(name)

        acc_max = singles.tile([P, 1], F32, name="acc_max_sb")
        acc_zsq = singles.tile([NZ, 1], F32, name="acc_zsq_sb")
        nc.vector.memset(acc_max, 0.0)
        nc.vector.memset(acc_zsq, 0.0)

        ctr = [0]

        def copy_act(dst, src, bias_ap, relu):
            # psum -> sbuf with per-partition bias (+ optional relu), engine
            # alternated between ACT and DVE so neither becomes the bottleneck
            if ctr[0] % 2 == 0:
                nc.scalar.activation(dst, src, Relu if relu else Ident,
                                     bias=bias_ap)
            else:
                if relu:
                    nc.vector.tensor_scalar(dst, src, bias_ap, 0.0, Add, Max)
                else:
                    nc.vector.tensor_scalar(dst, src, bias_ap, None, Add)
            ctr[0] += 1

        def layer(dst, mt, src, kt, wname, bname, relu):
            # dst [128, mt, NB] sbuf <- act(w.T @ src + b); src [128, kt, NB]
            wt, bt = w[wname], bias[bname]
            for m in range(mt):
                ps = psum.tile([P, NB], F32, tag="ps", name="ps")
                for k in range(kt):
                    nc.tensor.matmul(ps, wt[:, k, m * P:(m + 1) * P],
                                     src[:, k, :], start=(k == 0),
                                     stop=(k == kt - 1))
                copy_act(dst[:, m, :], ps, bt[:, m:m + 1], relu)

        def enc12(h, sfx):
            e1 = mid.tile([P, H // P, NB], BF16_DT, tag="mid", name="e1")
            layer(e1, H // P, h, H // P, f"w_e1{sfx}", f"b_e1{sfx}", True)
            e2 = big.tile([P, E3 // P, NB], BF16_DT, tag="big", name="e2")
            layer(e2, E3 // P, e1, H // P, f"w_e2{sfx}", f"b_e2{sfx}", True)
            return e2

        def enc3_packed(e2s, e2o):
            # Both final 2048->64 linears have M=64, so they run packed in
            # disjoint PE column halves (tile_position col groups) and their
            # matmul pairs execute concurrently: ~16 matmul slots instead
            # of 32.  Separate psum tiles keep the accumulation groups
            # independent; the (0,64) output must sit at base partition 64.
            ps_s = psum.tile([P, NB], F32, tag="ps", name="ps")
            ps_o = psum.tile([P, NB], F32, tag="ps", name="ps")
            ws, wo = w["w_e3s"], w["w_e3o"]
            for k in range(E3 // P):
                nc.tensor.matmul(ps_s[:NZ, :], ws[:, k, :], e2s[:, k, :],
                                 start=(k == 0), stop=(k == E3 // P - 1),
                                 tile_position=(0, 0))
                nc.tensor.matmul(ps_o[NZ:2 * NZ, :], wo[:, k, :], e2o[:, k, :],
                                 start=(k == 0), stop=(k == E3 // P - 1),
                                 tile_position=(0, NZ))
            z_s = zp.tile([NZ, NB], F32, tag="z", name="z_s")
            copy_act(z_s, ps_s[:NZ, :], bias["b_e3s"][:, 0:1], False)
            z_p = zp.tile([NZ, NB], F32, tag="z", name="z_p")
            copy_act(z_p, ps_o[NZ:2 * NZ, :], bias["b_e3o"][:, 0:1], False)
            return z_s, z_p

        chunk_list = [cc for _ in range(repeat) for cc in range(nchunk)]
        for ci_, c in enumerate(chunk_list):
            last_chunk = ci_ == len(chunk_list) - 1
            # stores go via SWDGE so the in-order SP (load) ring is never
            # blocked behind them; last chunk uses the now-idle SP ring to
            # shorten the kernel tail drain
            st_eng = nc.sync if last_chunk else nc.gpsimd
            bs = slice(c * NB, (c + 1) * NB)

            # ---- input layer: stream x in 4 batched sub-tiles, 4 dedicated
            # psum banks (pool psin) accumulate over all 16 k-tiles ----
            pss = [psin.tile([P, NB], F32, tag="psi", name="psi")
                   for _ in range(H // P)]
            pre = preloaded.pop(c, None)
            for kb in range(KX // P // KSUB):
                if pre is not None:
                    xt = pre[kb]
                else:
                    xt = xin.tile([P, KSUB, NB], BF16_DT, tag="xt", name="xt")
                    nc.sync.dma_start(out=xt,
                                      in_=x_R[:, kb * KSUB:(kb + 1) * KSUB, bs])
                for j in range(KSUB):
                    k = kb * KSUB + j
                    for m in range(H // P):
                        nc.tensor.matmul(pss[m],
                                         w_in_parts[k // 4][:, k % 4,
                                                            m * P:(m + 1) * P],
                                         xt[:, j, :], start=(k == 0),
                                         stop=(k == KX // P - 1))
            h = hp.tile([P, H // P, NB], BF16_DT, tag="h", name="h")
            for m in range(H // P):
                copy_act(h[:, m, :], pss[m], bias["b_in"][:, m:m + 1], True)

            # ---- encoders: e1/e2 of both, then the packed e3 pair ----
            e2s = enc12(h, "s")
            e2o = enc12(h, "o")
            z_s, z_p = enc3_packed(e2s, e2o)
            st_eng.dma_start(out=z_s_T[:, bs], in_=z_s)
            st_eng.dma_start(out=z_p_T[:, bs], in_=z_p)

            z_aug = zmisc.tile([P, NB], BF16_DT, tag="zaug", name="z_aug")
            nc.vector.memset(z_aug[NZ:, :], 0.0)
            nc.vector.memset(z_aug[NZ:NZ + 1, :], 1.0)
            nc.vector.tensor_scalar_mul(z_aug[:NZ, :], z_s, -2.0)
            lm = qp.tile([P, NB // P, CT], F32, tag="lm", name="lm")
            zsq = zmisc.tile([NZ, NB], F32, tag="zsq", name="zsq")
            nc.vector.tensor_mul(zsq, z_s, z_s)
            rs = qp.tile([NZ, 1], F32, tag="rs", name="rs")
            nc.vector.reduce_sum(rs, zsq, axis=AX)
            nc.vector.tensor_add(acc_zsq, acc_zsq, rs)

            quant_list = [(mb, qq) for mb in range(NB // P) for qq in range(CT)]
            qi = [0]

            def emit_quant(n):
                # VQ distance matmul + DVE rowmax; each psum drains via a
                # DVE reduce (~3x a matmul) so these are spread as PE filler
                for _ in range(n):
                    if qi[0] >= len(quant_list):
                        return
                    mb, qq = quant_list[qi[0]]
                    qi[0] += 1
                    qps = psum.tile([P, NB], F32, tag="ps", name="ps")
                    nc.tensor.matmul(qps, z_aug[:, mb * P:(mb + 1) * P],
                                     cb[:, qq * NB:(qq + 1) * NB],
                                     start=True, stop=True)
                    nc.vector.reduce_max(lm[:, mb, qq:qq + 1], qps, axis=AX)

            # fill the z -> z_sum -> d1 dependency latency with VQ matmuls
            emit_quant(6)

            # ---- decoder ----
            z_sum = zmisc.tile([P, NB], BF16_DT, tag="zsum", name="z_sum")
            nc.vector.memset(z_sum[NZ:, :], 0.0)
            nc.vector.tensor_add(z_sum[:NZ, :], z_p, z_s)

            d1 = big.tile([P, E3 // P, NB], BF16_DT, tag="big", name="d1")
            wt, bt = w["w_d1"], bias["b_d1"]
            for m in range(E3 // P):
                ps = psum.tile([P, NB], F32, tag="ps", name="ps")
                nc.tensor.matmul(ps, wt[:, 0, m * P:(m + 1) * P], z_sum,
                                 start=True, stop=True)
                copy_act(d1[:, m, :], ps, bt[:, m:m + 1], True)
                if m % 3 == 2:
                    emit_quant(1)

            # ---- d2 matmuls interleaved with remaining VQ matmuls ----
            d2 = mid.tile([P, H // P, NB], BF16_DT, tag="mid", name="d2")
            wt, bt = w["w_d2"], bias["b_d2"]
            for m in range(H // P):
                ps = psum.tile([P, NB], F32, tag="ps", name="ps")
                for k in range(E3 // P):
                    nc.tensor.matmul(ps, wt[:, k, m * P:(m + 1) * P],
                                     d1[:, k, :], start=(k == 0),
                                     stop=(k == E3 // P - 1))
                    if k % 2 == 1:
                        emit_quant(1)
                copy_act(d2[:, m, :], ps, bt[:, m:m + 1], True)
            assert qi[0] == len(quant_list)
            for mb in range(NB // P):
                rm = qp.tile([P, 1], F32, tag="rm", name="rm")
                nc.vector.reduce_max(rm, lm[:, mb, :], axis=AX)
                nc.vector.tensor_add(acc_max, acc_max, rm)

            d3 = mid.tile([P, H // P, NB], BF16_DT, tag="mid", name="d3")
            layer(d3, H // P, d2, H // P, "w_d3", "b_d3", True)

            # ---- head: fp32 out, DMA per m-tile ----
            wt, bt = w["w_hd"], bias["b_hd"]
            for m in range(OUT // P):
                ps = psum.tile([P, NB], F32, tag="ps", name="ps")
                for k in range(H // P):
                    nc.tensor.matmul(ps, wt[:, k, m * P:(m + 1) * P],
                                     d3[:, k, :], start=(k == 0),
                                     stop=(k == H // P - 1))
                ho = hout.tile([P, NB], F32, tag="ho", name="ho")
                copy_act(ho, ps, bt[:, m:m + 1], False)
                st_eng.dma_start(out=xhat_T[m * P:(m + 1) * P, bs], in_=ho)

        nc.sync.dma_start(out=acc_max_d, in_=acc_max)
        nc.sync.dma_start(out=acc_zsq_d, in_=acc_zsq)


# ---------------- host-side data prep ----------------

def _tile_w(W, Kp, Mp):
    W = np.asarray(W, np.float32)
    K, M = W.shape
    Wp = np.zeros((Kp, Mp), np.float32)
    Wp[:K, :M] = W
    return Wp.reshape(Kp // P, P, Mp).transpose(1, 0, 2).astype(BF16)


def _tile_b(bv, Mp):
    bv = np.asarray(bv, np.float32)
    bp = np.zeros((Mp,), np.float32)
    bp[:bv.shape[0]] = bv
    if Mp < P:
        return np.ascontiguousarray(bp.reshape(1, Mp).T)
    return np.ascontiguousarray(bp.reshape(Mp // P, P).T)


def _prep_mod_inputs(params, modality):
    own_in = params["in_m"] if modality == "m" else params["in_a"]
    enc_own = params["enc_main"] if modality == "m" else params["enc_aux"]
    enc_sh = params["enc_shared"]
    dec = params["dec_shared"]
    head = params["dec_m"] if modality == "m" else params["dec_a"]
    cb = np.asarray(params["codebook"], np.float32)

    m = {}
    m["w_in"], m["b_in"] = _tile_w(own_in[0], KX, H), _tile_b(own_in[1], H)
    for sfx, enc in (("o", enc_own), ("s", enc_sh)):
        m[f"w_e1{sfx}"] = _tile_w(enc[0][0], H, H)
        m[f"b_e1{sfx}"] = _tile_b(enc[0][1], H)
        m[f"w_e2{sfx}"] = _tile_w(enc[1][0], H, E3)
        m[f"b_e2{sfx}"] = _tile_b(enc[1][1], E3)
        m[f"w_e3{sfx}"] = _tile_w(enc[2][0], E3, NZ)
        m[f"b_e3{sfx}"] = _tile_b(enc[2][1], NZ)
    m["w_d1"], m["b_d1"] = _tile_w(dec[0][0], P, E3), _tile_b(dec[0][1], E3)
    m["w_d2"], m["b_d2"] = _tile_w(dec[1][0], E3, H), _tile_b(dec[1][1], H)
    m["w_d3"], m["b_d3"] = _tile_w(dec[2][0], H, H), _tile_b(dec[2][1], H)
    m["w_hd"], m["b_hd"] = _tile_w(head[0], H, OUT), _tile_b(head[1], OUT)

    cb_aug = np.zeros((P, C), np.float32)
    cb_aug[:NZ, :] = cb.T
    cb_aug[NZ, :] = (cb * cb).sum(axis=1)
    m["cb_aug"] = cb_aug.astype(BF16)
    return m


def _prep_x(x, shard):
    xs = np.asarray(x[shard * B_CORE:(shard + 1) * B_CORE], np.float32)
    xp = np.zeros((KX, B_CORE), np.float32)
    xp[:NX, :] = xs.T
    return xp.astype(BF16)


_CACHE = {}


def _get_program():
    if "nc" not in _CACHE:
        _CACHE["nc"] = build_program()
    return _CACHE["nc"]


def build_in_maps(x_m, x_a, params):
    mod_inputs = {mod: _prep_mod_inputs(params, mod) for mod in ("m", "a")}
    in_maps = []
    for core in range(N_CORES):
        mod = "m" if core < CORES_PER_MOD else "a"
        im = dict(mod_inputs[mod])
        im["x_T"] = _prep_x(x_m if mod == "m" else x_a, core % CORES_PER_MOD)
        in_maps.append(im)
    return in_maps


def gather(outs):
    # outs: list (per core) of dicts name -> np.ndarray
    def gather_z(name, lo, hi):
        return np.concatenate(
            [outs[c][name].T for c in range(lo, hi)], axis=0).astype(np.float32)

    z_m_p = gather_z("z_p_T", 0, CORES_PER_MOD)
    z_m_s = gather_z("z_s_T", 0, CORES_PER_MOD)
    z_a_p = gather_z("z_p_T", CORES_PER_MOD, N_CORES)
    z_a_s = gather_z("z_s_T", CORES_PER_MOD, N_CORES)
    x_m_hat = np.concatenate(
        [outs[c]["xhat_T"][:NOUT].T for c in range(CORES_PER_MOD)], axis=0)
    x_a_hat = np.concatenate(
        [outs[c]["xhat_T"][:NOUT].T for c in range(CORES_PER_MOD, N_CORES)],
        axis=0)
    total = 0.0
    for c in range(N_CORES):
        total += float(outs[c]["acc_max"].sum()) + float(outs[c]["acc_zsq"].sum())
    loss = np.float32((1.0 + BETA) / (B_FULL * NZ) * total)

    return ((z_m_p, z_m_s, z_a_p, z_a_s),
            (np.ascontiguousarray(x_m_hat), np.ascontiguousarray(x_a_hat)),
            loss)


def run(x_m, x_a, params, trace=False):
    nc = _get_program()
    in_maps = build_in_maps(x_m, x_a, params)
    res = bass_utils.run_bass_kernel_spmd(
        nc, in_maps, core_ids=list(range(N_CORES)), trace=trace)
    return gather(res.results), res


def kernel(x_m, x_a, params):
    result, _ = run(x_m, x_a, params)
    return result


# revision 24
# speedup vs baseline: 7.5472x; 1.3167x over previous
# Trainium2 Bass kernel for nn_DecomposeNetwork (vq_codebook).
#
# Reference computation (per modality x in {m, a}):
#   h   = relu(x @ W_in + b)                       [B, 500]
#   z_p = enc_own(h)    (500->500->2000->64, relu on first two)
#   z_s = enc_shared(h) (same shapes)
#   loss term from VQ distances of z_s vs codebook [4096, 64]
#   z   = z_p + z_s
#   xh  = dec_head(dec_shared(z))                  [B, 2000]
# Returned: ((z_m_p, z_m_s, z_a_p, z_a_s), (x_m_hat, x_a_hat), loss_code)
#
# Key simplification: the quantized vectors / soft-assignment q are dead code
# for the returned outputs.  loss = 1.25/(B*64) * sum_b max_c d[b,c] where
# d[b,c] = ||z_b||^2 + ||cb_c||^2 - 2 z_b.cb_c  (argmax ties don't change the
# max value).  We compute max_c(||cb_c||^2 - 2 z.cb) via a matmul with an
# augmented codebook row, plus sum ||z||^2, accumulated per-core and reduced
# on the host (the only cross-device term).
#
# Sharding: cores 0-3 process x_m (4096 rows each), cores 4-7 process x_a.
# The modality split halves the per-core weight footprint so ALL weights stay
# resident in SBUF for the whole kernel.  All cores run one SPMD program;
# per-core differences are input bindings only.
#
# Layout: activations are feature-major [features(partitions), batch(free)];
# weights are host-pre-tiled to [128, k_tiles, M] so each [128,128] stationary
# tile is a direct slice.  Dims padded: 2000->2048, 500->512.  Matmuls in
# bf16 with fp32 PSUM accumulation; outputs stored fp32.

import numpy as np
import ml_dtypes

import concourse.bass as bass
import concourse.tile as tile
from concourse import bacc
from concourse import mybir
from concourse import bass_utils

BF16 = ml_dtypes.bfloat16

P = 128
NB = 512          # batch columns per chunk (matmul free dim / PSUM bank)
B_FULL = 16384
N_CORES = 8
CORES_PER_MOD = 4
B_CORE = B_FULL // CORES_PER_MOD   # 4096 rows per core

NX = 2000         # raw input features
KX = 2048         # padded input features  (16 k-tiles)
NH = 500
H = 512           # padded hidden          (4 tiles)
NE3 = 2000
E3 = 2048         # padded encoder L2 out  (16 tiles)
NZ = 64
C = 4096          # codebook entries
CT = C // NB      # codebook column chunks (8)
NOUT = 2000
OUT = 2048        # padded output features (16 tiles)
BETA = 0.25

F32 = mybir.dt.float32
BF16_DT = mybir.dt.bfloat16

# name -> (k_tiles, M) for the [128, k_tiles, M] weight layout
WEIGHT_SPECS = {
    "w_in":  (KX // P, H),
    "w_e1o": (H // P, H),
    "w_e2o": (H // P, E3),
    "w_e3o": (E3 // P, NZ),
    "w_e1s": (H // P, H),
    "w_e2s": (H // P, E3),
    "w_e3s": (E3 // P, NZ),
    "w_d1":  (1, E3),          # K=64 zero-padded to 128
    "w_d2":  (E3 // P, H),
    "w_d3":  (H // P, H),
    "w_hd":  (H // P, OUT),
}
# name -> (partitions, m_tiles)
BIAS_SPECS = {
    "b_in":  (P, H // P),
    "b_e1o": (P, H // P),
    "b_e2o": (P, E3 // P),
    "b_e3o": (NZ, 1),
    "b_e1s": (P, H // P),
    "b_e2s": (P, E3 // P),
    "b_e3s": (NZ, 1),
    "b_d1":  (P, E3 // P),
    "b_d2":  (P, H // P),
    "b_d3":  (P, H // P),
    "b_hd":  (P, OUT // P),
}


def build_program(b_core=B_CORE, num_devices=N_CORES, repeat=1):
    nc = bacc.Bacc("TRN2", debug=False, num_devices=num_devices)

    x_T = nc.dram_tensor("x_T", [KX, b_core], BF16_DT, kind="ExternalInput").ap()
    w = {
        name: nc.dram_tensor(name, [P, kt, m], BF16_DT, kind="ExternalInput").ap()
        for name, (kt, m) in WEIGHT_SPECS.items()
    }
    b = {
        name: nc.dram_tensor(name, [pp, mt], F32, kind="ExternalInput").ap()
        for name, (pp, mt) in BIAS_SPECS.items()
    }
    cb_aug = nc.dram_tensor("cb_aug", [P, C], BF16_DT, kind="ExternalInput").ap()

    z_p_T = nc.dram_tensor("z_p_T", [NZ, b_core], F32, kind="ExternalOutput").ap()
    z_s_T = nc.dram_tensor("z_s_T", [NZ, b_core], F32, kind="ExternalOutput").ap()
    xhat_T = nc.dram_tensor("xhat_T", [OUT, b_core], F32, kind="ExternalOutput").ap()
    acc_max_d = nc.dram_tensor("acc_max", [P, 1], F32, kind="ExternalOutput").ap()
    acc_zsq_d = nc.dram_tensor("acc_zsq", [NZ, 1], F32, kind="ExternalOutput").ap()

    with tile.TileContext(nc) as tc:
        _emit(tc, b_core, x_T, w, b, cb_aug, z_p_T, z_s_T, xhat_T,
              acc_max_d, acc_zsq_d, repeat=repeat)
    nc.compile()
    return nc


def _emit(tc, b_core, x_T, w_d, b_d, cb_d, z_p_T, z_s_T, xhat_T,
          acc_max_d, acc_zsq_d, repeat=1):
    nc = tc.nc
    nchunk = b_core // NB
    x_R = x_T.rearrange("(kt p) b -> p kt b", p=P)
    Relu = mybir.ActivationFunctionType.Relu
    Ident = mybir.ActivationFunctionType.Identity
    Add = mybir.AluOpType.add
    Max = mybir.AluOpType.max
    AX = mybir.AxisListType.X
    KSUB = 4

    with (
        tc.tile_pool(name="singles", bufs=1) as singles,
        tc.tile_pool(name="xin", bufs=4) as xin,
        tc.tile_pool(name="hp", bufs=2) as hp,
        tc.tile_pool(name="mid", bufs=3) as mid,
        tc.tile_pool(name="big", bufs=2) as big,
        tc.tile_pool(name="zp", bufs=3) as zp,
        tc.tile_pool(name="zmisc", bufs=2) as zmisc,
        tc.tile_pool(name="hout", bufs=3) as hout,
        tc.tile_pool(name="qp", bufs=4) as qp,
        tc.tile_pool(name="psum", bufs=4, space="PSUM") as psum,
        tc.tile_pool(name="psin", bufs=4, space="PSUM") as psin,
    ):
        # ---- resident weights / biases ----
        # The SP HWDGE ring is in-order, so DMA emission order is queue
        # order: first-chunk inputs (w_in, b_in, x chunk 0) go first so PE
        # can start early; remaining weights stream in usage order.
        w = {}
        bias = {}

        def load_w(name):
            ap = w_d[name]
            t = singles.tile(list(ap.shape), BF16_DT, name=f"sb_{name}")
            nc.sync.dma_start(out=t, in_=ap)
            w[name] = t

        def load_b(name):
            ap = b_d[name]
            t = singles.tile(list(ap.shape), F32, name=f"sb_{name}")
            nc.sync.dma_start(out=t, in_=ap)
            bias[name] = t

        # w_in split 4 ways: the very first LDWEIGHTS then waits on a
        # 0.5MB DMA instead of the full 2MB
        w_in_parts = []
        for i in range(4):
            t = singles.tile([P, 4, H], BF16_DT, name=f"sb_w_in_{i}")
            nc.sync.dma_start(out=t, in_=w_d["w_in"][:, 4 * i:4 * (i + 1), :])
            w_in_parts.append(t)
        load_b("b_in")
        x0_tiles = []
        for kb in range(KX // P // KSUB):
            xt = xin.tile([P, KSUB, NB], BF16_DT, tag="xt", name="xt")
            nc.sync.dma_start(out=xt, in_=x_R[:, kb * KSUB:(kb + 1) * KSUB, 0:NB])
            x0_tiles.append(xt)
        preloaded = {0: x0_tiles}
        for name in ("b_e1s", "b_e2s", "b_e3s", "b_e1o", "b_e2o", "b_e3o",
                     "b_d1", "b_d2", "b_d3", "b_hd"):
            load_b(name)
        for name in ("w_e1s", "w_e2s", "w_e1o", "w_e2o", "w_e3s", "w_e3o"):
            load_w(name)
        cb = singles.tile([P, C], BF16_DT, name="sb_cb")
        nc.sync.dma_start(out=cb, in_=cb_d)
        for name in ("w_d1", "w_d2", "w_d3", "w_hd"):
            load_w(name)

        acc_max = singles.tile([P, 1], F32, name="acc_max_sb")
        acc_zsq = singles.tile([NZ, 1], F32, name="acc_zsq_sb")
        nc.vector.memset(acc_max, 0.0)
        nc.vector.memset(acc_zsq, 0.0)

        ctr = [0]

        def copy_act(dst, src, bias_ap, relu):
            # psum -> sbuf with per-partition bias (+ optional relu), split
            # 2/3 ACT : 1/3 DVE — DVE also carries the VQ reduce_max drains,
            # so it gets the smaller share
            if ctr[0] % 3 != 0:
                nc.scalar.activation(dst, src, Relu if relu else Ident,
                                     bias=bias_ap)
            else:
                if relu:
                    nc.vector.tensor_scalar(dst, src, bias_ap, 0.0, Add, Max)
                else:
                    nc.vector.tensor_scalar(dst, src, bias_ap, None, Add)
            ctr[0] += 1

        def layer(dst, mt, src, kt, wname, bname, relu):
            # dst [128, mt, NB] sbuf <- act(w.T @ src + b); src [128, kt, NB]
            wt, bt = w[wname], bias[bname]
            for m in range(mt):
                ps = psum.tile([P, NB], F32, tag="ps", name="ps")
                for k in range(kt):
                    nc.tensor.matmul(ps, wt[:, k, m * P:(m + 1) * P],
                                     src[:, k, :], start=(k == 0),
                                     stop=(k == kt - 1))
                copy_act(dst[:, m, :], ps, bt[:, m:m + 1], relu)

        def enc12(h, sfx):
            e1 = mid.tile([P, H // P, NB], BF16_DT, tag="mid", name="e1")
            layer(e1, H // P, h, H // P, f"w_e1{sfx}", f"b_e1{sfx}", True)
            e2 = big.tile([P, E3 // P, NB], BF16_DT, tag="big", name="e2")
            layer(e2, E3 // P, e1, H // P, f"w_e2{sfx}", f"b_e2{sfx}", True)
            return e2

        def enc3_packed(e2s, e2o):
            # Both final 2048->64 linears have M=64, so they run packed in
            # disjoint PE column halves (tile_position col groups) and their
            # matmul pairs execute concurrently: ~16 matmul slots instead
            # of 32.  Separate psum tiles keep the accumulation groups
            # independent; the (0,64) output must sit at base partition 64.
            ps_s = psum.tile([P, NB], F32, tag="ps", name="ps")
            ps_o = psum.tile([P, NB], F32, tag="ps", name="ps")
            ws, wo = w["w_e3s"], w["w_e3o"]
            for k in range(E3 // P):
                nc.tensor.matmul(ps_s[:NZ, :], ws[:, k, :], e2s[:, k, :],
                                 start=(k == 0), stop=(k == E3 // P - 1),
                                 tile_position=(0, 0))
                nc.tensor.matmul(ps_o[NZ:2 * NZ, :], wo[:, k, :], e2o[:, k, :],
                                 start=(k == 0), stop=(k == E3 // P - 1),
                                 tile_position=(0, NZ))
            z_s = zp.tile([NZ, NB], F32, tag="z", name="z_s")
            copy_act(z_s, ps_s[:NZ, :], bias["b_e3s"][:, 0:1], False)
            z_p = zp.tile([NZ, NB], F32, tag="z", name="z_p")
            copy_act(z_p, ps_o[NZ:2 * NZ, :], bias["b_e3o"][:, 0:1], False)
            return z_s, z_p

        chunk_list = [cc for _ in range(repeat) for cc in range(nchunk)]
        for ci_, c in enumerate(chunk_list):
            last_chunk = ci_ == len(chunk_list) - 1
            # stores go via SWDGE so the in-order SP (load) ring is never
            # blocked behind them; last chunk uses the now-idle SP ring to
            # shorten the kernel tail drain
            st_eng = nc.sync if last_chunk else nc.gpsimd
            bs = slice(c * NB, (c + 1) * NB)

            # ---- input layer: stream x in 4 batched sub-tiles, 4 dedicated
            # psum banks (pool psin) accumulate over all 16 k-tiles ----
            pss = [psin.tile([P, NB], F32, tag="psi", name="psi")
                   for _ in range(H // P)]
            pre = preloaded.pop(c, None)
            for kb in range(KX // P // KSUB):
                if pre is not None:
                    xt = pre[kb]
                else:
                    xt = xin.tile([P, KSUB, NB], BF16_DT, tag="xt", name="xt")
                    nc.sync.dma_start(out=xt,
                                      in_=x_R[:, kb * KSUB:(kb + 1) * KSUB, bs])
                for j in range(KSUB):
                    k = kb * KSUB + j
                    for m in range(H // P):
                        nc.tensor.matmul(pss[m],
                                         w_in_parts[k // 4][:, k % 4,
                                                            m * P:(m + 1) * P],
                                         xt[:, j, :], start=(k == 0),
                                         stop=(k == KX // P - 1))
            h = hp.tile([P, H // P, NB], BF16_DT, tag="h", name="h")
            for m in range(H // P):
                copy_act(h[:, m, :], pss[m], bias["b_in"][:, m:m + 1], True)

            # ---- encoders: e1/e2 of both, then the packed e3 pair ----
            e2s = enc12(h, "s")
            e2o = enc12(h, "o")
            z_s, z_p = enc3_packed(e2s, e2o)
            st_eng.dma_start(out=z_s_T[:, bs], in_=z_s)
            st_eng.dma_start(out=z_p_T[:, bs], in_=z_p)

            z_aug = zmisc.tile([P, NB], BF16_DT, tag="zaug", name="z_aug")
            nc.vector.memset(z_aug[NZ:, :], 0.0)
            nc.vector.memset(z_aug[NZ:NZ + 1, :], 1.0)
            nc.vector.tensor_scalar_mul(z_aug[:NZ, :], z_s, -2.0)
            lm = qp.tile([P, NB // P, CT], F32, tag="lm", name="lm")
            zsq = zmisc.tile([NZ, NB], F32, tag="zsq", name="zsq")
            nc.vector.tensor_mul(zsq, z_s, z_s)
            rs = qp.tile([NZ, 1], F32, tag="rs", name="rs")
            nc.vector.reduce_sum(rs, zsq, axis=AX)
            nc.vector.tensor_add(acc_zsq, acc_zsq, rs)

            quant_list = [(mb, qq) for mb in range(NB // P) for qq in range(CT)]
            qi = [0]

            def emit_quant(n):
                # VQ distance matmul + DVE rowmax; each psum drains via a
                # DVE reduce (~3x a matmul) so these are spread as PE filler
                for _ in range(n):
                    if qi[0] >= len(quant_list):
                        return
                    mb, qq = quant_list[qi[0]]
                    qi[0] += 1
                    qps = psum.tile([P, NB], F32, tag="ps", name="ps")
                    nc.tensor.matmul(qps, z_aug[:, mb * P:(mb + 1) * P],
                                     cb[:, qq * NB:(qq + 1) * NB],
                                     start=True, stop=True)
                    nc.vector.reduce_max(lm[:, mb, qq:qq + 1], qps, axis=AX)

            # fill the z -> z_sum -> d1 dependency latency with VQ matmuls
            emit_quant(6)

            # ---- decoder ----
            z_sum = zmisc.tile([P, NB], BF16_DT, tag="zsum", name="z_sum")
            nc.vector.memset(z_sum[NZ:, :], 0.0)
            nc.vector.tensor_add(z_sum[:NZ, :], z_p, z_s)

            d1 = big.tile([P, E3 // P, NB], BF16_DT, tag="big", name="d1")
            wt, bt = w["w_d1"], bias["b_d1"]
            for m in range(E3 // P):
                ps = psum.tile([P, NB], F32, tag="ps", name="ps")
                nc.tensor.matmul(ps, wt[:, 0, m * P:(m + 1) * P], z_sum,
                                 start=True, stop=True)
                copy_act(d1[:, m, :], ps, bt[:, m:m + 1], True)
                if m % 3 == 2:
                    emit_quant(1)

            # ---- d2 matmuls interleaved with remaining VQ matmuls ----
            d2 = mid.tile([P, H // P, NB], BF16_DT, tag="mid", name="d2")
            wt, bt = w["w_d2"], bias["b_d2"]
            for m in range(H // P):
                ps = psum.tile([P, NB], F32, tag="ps", name="ps")
                for k in range(E3 // P):
                    nc.tensor.matmul(ps, wt[:, k, m * P:(m + 1) * P],
                                     d1[:, k, :], start=(k == 0),
                                     stop=(k == E3 // P - 1))
                    if k % 2 == 1:
                        emit_quant(1)
                copy_act(d2[:, m, :], ps, bt[:, m:m + 1], True)
            assert qi[0] == len(quant_list)
            for mb in range(NB // P):
                rm = qp.tile([P, 1], F32, tag="rm", name="rm")
                nc.vector.reduce_max(rm, lm[:, mb, :], axis=AX)
                nc.vector.tensor_add(acc_max, acc_max, rm)

            d3 = mid.tile([P, H // P, NB], BF16_DT, tag="mid", name="d3")
            layer(d3, H // P, d2, H // P, "w_d3", "b_d3", True)

            # ---- head: fp32 out, DMA per m-tile ----
            wt, bt = w["w_hd"], bias["b_hd"]
            for m in range(OUT // P):
                ps = psum.tile([P, NB], F32, tag="ps", name="ps")
                for k in range(H // P):
                    nc.tensor.matmul(ps, wt[:, k, m * P:(m + 1) * P],
                                     d3[:, k, :], start=(k == 0),
                                     stop=(k == H // P - 1))
                ho = hout.tile([P, NB], F32, tag="ho", name="ho")
                copy_act(ho, ps, bt[:, m:m + 1], False)
                st_eng.dma_start(out=xhat_T[m * P:(m + 1) * P, bs], in_=ho)

        nc.sync.dma_start(out=acc_max_d, in_=acc_max)
        nc.sync.dma_start(out=acc_zsq_d, in_=acc_zsq)


# ---------------- host-side data prep ----------------

def _tile_w(W, Kp, Mp):
    W = np.asarray(W, np.float32)
    K, M = W.shape
    Wp = np.zeros((Kp, Mp), np.float32)
    Wp[:K, :M] = W
    return Wp.reshape(Kp // P, P, Mp).transpose(1, 0, 2).astype(BF16)


def _tile_b(bv, Mp):
    bv = np.asarray(bv, np.float32)
    bp = np.zeros((Mp,), np.float32)
    bp[:bv.shape[0]] = bv
    if Mp < P:
        return np.ascontiguousarray(bp.reshape(1, Mp).T)
    return np.ascontiguousarray(bp.reshape(Mp // P, P).T)


def _prep_mod_inputs(params, modality):
    own_in = params["in_m"] if modality == "m" else params["in_a"]
    enc_own = params["enc_main"] if modality == "m" else params["enc_aux"]
    enc_sh = params["enc_shared"]
    dec = params["dec_shared"]
    head = params["dec_m"] if modality == "m" else params["dec_a"]
    cb = np.asarray(params["codebook"], np.float32)

    m = {}
    m["w_in"], m["b_in"] = _tile_w(own_in[0], KX, H), _tile_b(own_in[1], H)
    for sfx, enc in (("o", enc_own), ("s", enc_sh)):
        m[f"w_e1{sfx}"] = _tile_w(enc[0][0], H, H)
        m[f"b_e1{sfx}"] = _tile_b(enc[0][1], H)
        m[f"w_e2{sfx}"] = _tile_w(enc[1][0], H, E3)
        m[f"b_e2{sfx}"] = _tile_b(enc[1][1], E3)
        m[f"w_e3{sfx}"] = _tile_w(enc[2][0], E3, NZ)
        m[f"b_e3{sfx}"] = _tile_b(enc[2][1], NZ)
    m["w_d1"], m["b_d1"] = _tile_w(dec[0][0], P, E3), _tile_b(dec[0][1], E3)
    m["w_d2"], m["b_d2"] = _tile_w(dec[1][0], E3, H), _tile_b(dec[1][1], H)
    m["w_d3"], m["b_d3"] = _tile_w(dec[2][0], H, H), _tile_b(dec[2][1], H)
    m["w_hd"], m["b_hd"] = _tile_w(head[0], H, OUT), _tile_b(head[1], OUT)

    cb_aug = np.zeros((P, C), np.float32)
    cb_aug[:NZ, :] = cb.T
    cb_aug[NZ, :] = (cb * cb).sum(axis=1)
    m["cb_aug"] = cb_aug.astype(BF16)
    return m


def _prep_x(x, shard):
    xs = np.asarray(x[shard * B_CORE:(shard + 1) * B_CORE], np.float32)
    xp = np.zeros((KX, B_CORE), np.float32)
    xp[:NX, :] = xs.T
    return xp.astype(BF16)


_CACHE = {}


def _get_program():
    if "nc" not in _CACHE:
        _CACHE["nc"] = build_program()
    return _CACHE["nc"]


def build_in_maps(x_m, x_a, params):
    mod_inputs = {mod: _prep_mod_inputs(params, mod) for mod in ("m", "a")}
    in_maps = []
    for core in range(N_CORES):
        mod = "m" if core < CORES_PER_MOD else "a"
        im = dict(mod_inputs[mod])
        im["x_T"] = _prep_x(x_m if mod == "m" else x_a, core % CORES_PER_MOD)
        in_maps.append(im)
    return in_maps


def gather(outs):
    # outs: list (per core) of dicts name -> np.ndarray
    def gather_z(name, lo, hi):
        return np.concatenate(
            [outs[c][name].T for c in range(lo, hi)], axis=0).astype(np.float32)

    z_m_p = gather_z("z_p_T", 0, CORES_PER_MOD)
    z_m_s = gather_z("z_s_T", 0, CORES_PER_MOD)
    z_a_p = gather_z("z_p_T", CORES_PER_MOD, N_CORES)
    z_a_s = gather_z("z_s_T", CORES_PER_MOD, N_CORES)
    x_m_hat = np.concatenate(
        [outs[c]["xhat_T"][:NOUT].T for c in range(CORES_PER_MOD)], axis=0)
    x_a_hat = np.concatenate(
        [outs[c]["xhat_T"][:NOUT].T for c in range(CORES_PER_MOD, N_CORES)],
        axis=0)
    total = 0.0
    for c in range(N_CORES):
        total += float(outs[c]["acc_max"].sum()) + float(outs[c]["acc_zsq"].sum())
    loss = np.float32((1.0 + BETA) / (B_FULL * NZ) * total)

    return ((z_m_p, z_m_s, z_a_p, z_a_s),
            (np.ascontiguousarray(x_m_hat), np.ascontiguousarray(x_a_hat)),
            loss)


def run(x_m, x_a, params, trace=False):
    nc = _get_program()
    in_maps = build_in_maps(x_m, x_a, params)
    res = bass_utils.run_bass_kernel_spmd(
        nc, in_maps, core_ids=list(range(N_CORES)), trace=trace)
    return gather(res.results), res


def kernel(x_m, x_a, params):
    result, _ = run(x_m, x_a, params)
    return result
